# revision 1
# baseline (speedup 1.0000x reference)
"""HGT link predictor on 8 trn2 NeuronCores.

Sharding: nodes split 8 ways per type (2500/core, padded to 2560).
Params replicated. Edges partitioned by destination core, sorted by dst,
packed into 128-edge chunks within 128-dst-node windows. Per layer:
local k_r/v_r projections (relation transforms folded into the weights on
host) -> AllGather -> per-edge dma_gather of k/v (+q) -> logits via
multiply+segmented reduce -> exp -> scatter-add via one-hot matmuls into
PSUM (softmax denominator rides as a 257th..264th column) -> normalize ->
gelu -> output linear -> gated skip + residual + LayerNorm + relu.
"""
import math
import numpy as np

import concourse.bacc as bacc
import concourse.bass as bass
import concourse.mybir as mybir
import concourse.tile as tile
from concourse.bass_utils import run_bass_kernel_spmd
from concourse.library_config import mlp

F32 = mybir.dt.float32
I16 = mybir.dt.int16
AF = mybir.ActivationFunctionType
OP = mybir.AluOpType

T, R, L = 3, 4, 2
H, HEADS, D, FIN, OUT = 256, 8, 32, 128, 128
SRC_T = (0, 1, 1, 1)
DST_T = (1, 0, 1, 2)
LN_EPS = 1e-5
NC = 8
N = 20000
NL = N // NC          # 2500 real local nodes per type
NT = 20               # node tiles of 128
NLP = NT * 128        # 2560 padded local nodes
NWIN = NT             # dst windows of 128 local nodes
GWIN = 2              # windows per gather group
KV_W = 2 * R * H      # 2048: [k0 v0 k1 v1 ...] columns of kv rows


def _block_diag(a):
    """a: [HEADS, D, D] -> [H, H] block diagonal."""
    out = np.zeros((H, H), np.float32)
    for h in range(HEADS):
        out[h * D:(h + 1) * D, h * D:(h + 1) * D] = a[h]
    return out


def _wrap_idx(idx):
    """idx [M] -> [128, M//16] int16 wrapped in 16 partitions, replicated."""
    m = idx.shape[0]
    assert m % 16 == 0
    w = np.zeros((128, m // 16), np.int16)
    w[:16] = idx.astype(np.int16).reshape(m // 16, 16).T
    for rep in range(1, 8):
        w[16 * rep:16 * rep + 16] = w[:16]
    return w


def _preprocess(inputs):
    x = np.asarray(inputs["x"], np.float32)
    edge_index = np.asarray(inputs["edge_index"])
    Win = np.asarray(inputs["Win"], np.float32)
    b_in = np.asarray(inputs["b_in"], np.float32)
    Wk = np.asarray(inputs["Wk"], np.float32); bk = np.asarray(inputs["bk"], np.float32)
    Wq = np.asarray(inputs["Wq"], np.float32); bq = np.asarray(inputs["bq"], np.float32)
    Wv = np.asarray(inputs["Wv"], np.float32); bv = np.asarray(inputs["bv"], np.float32)
    Wa = np.asarray(inputs["Wa"], np.float32); ba = np.asarray(inputs["ba"], np.float32)
    skip = np.asarray(inputs["skip"], np.float32)
    a_rel = np.asarray(inputs["a_rel"], np.float32)
    m_rel = np.asarray(inputs["m_rel"], np.float32)
    p_rel = np.asarray(inputs["p_rel"], np.float32)
    ln_g = np.asarray(inputs["ln_g"], np.float32)
    ln_b = np.asarray(inputs["ln_b"], np.float32)
    Wout = np.asarray(inputs["Wout"], np.float32)
    bout = np.asarray(inputs["bout"], np.float32)

    meta = {}
    # folded weights ------------------------------------------------------
    inv_sqrt_d = 1.0 / math.sqrt(D)
    wkr = np.zeros((L, R, H, H), np.float32); bkr = np.zeros((L, R, H), np.float32)
    wvr = np.zeros((L, R, H, H), np.float32); bvr = np.zeros((L, R, H), np.float32)
    for l in range(L):
        for r in range(R):
            st = SRC_T[r]
            ak = _block_diag(a_rel[l, r] * (p_rel[l, r] * inv_sqrt_d)[:, None, None])
            av = _block_diag(m_rel[l, r])
            wkr[l, r] = Wk[l, st] @ ak; bkr[l, r] = bk[l, st] @ ak
            wvr[l, r] = Wv[l, st] @ av; bvr[l, r] = bv[l, st] @ av
    beta = 1.0 / (1.0 + np.exp(-skip))          # [L, T]
    g = beta / (2.0 - beta)
    wa_eff = Wa * g[:, :, None, None]
    ba_eff = ba * g[:, :, None]
    meta["eps_eff"] = (LN_EPS / (2.0 - beta) ** 2).tolist()

    meta["use_bias"] = dict(
        bin_=bool(np.any(b_in)), bq=bool(np.any(bq)),
        bkr=bool(np.any(bkr)) or bool(np.any(bvr)),
        ba=bool(np.any(ba_eff)), bout=bool(np.any(bout)),
        lng=not np.allclose(ln_g, 1.0), lnb=bool(np.any(ln_b)),
    )

    def bcast(v):
        # [..., F] -> [..., 128, F]: per-feature vectors replicated across partitions
        return np.ascontiguousarray(
            np.broadcast_to(v[..., None, :], v.shape[:-1] + (128, v.shape[-1])))

    # edge partitioning ---------------------------------------------------
    per_core = []
    win_edges = [[] for _ in range(NC)]   # [c][r][w] -> (src_rows, dst_loc)
    kch_need = 1
    for c in range(NC):
        rel = []
        for r in range(R):
            src = edge_index[r, 0].astype(np.int64)
            dst = edge_index[r, 1].astype(np.int64)
            sel = (dst // NL) == c
            s, d = src[sel], dst[sel] - c * NL
            o = np.argsort(d, kind="stable")
            s, d = s[o], d[o]
            wins = []
            for w in range(NWIN):
                m = (d // 128) == w
                sw, dw = s[m], d[m]
                kch_need = max(kch_need, (len(sw) + 127) // 128)
                wins.append((sw, dw))
            rel.append(wins)
        win_edges[c] = rel
    KCH = kch_need
    meta["KCH"] = KCH
    NCHUNK = NWIN * KCH
    NIDX_R = NCHUNK * 128

    for c in range(NC):
        oh = np.zeros((R, NCHUNK, 128, 128), np.float32)
        kv_idx = np.zeros((R, NIDX_R), np.int64)
        qi_idx = np.zeros((R, NIDX_R), np.int64)
        for r in range(R):
            for w in range(NWIN):
                sw, dw = win_edges[c][r][w]
                ne = len(sw)
                base = w * KCH * 128
                # src node n (global) -> kv_full row (n//NL)*NLP + n%NL
                kv_idx[r, base:base + ne] = (sw // NL) * NLP + (sw % NL)
                qi_idx[r, base:base + ne] = dw
                ch = base // 128 + np.arange(ne) // 128
                oh[r, ch, np.arange(ne) % 128, dw - w * 128] = 1.0
        # partition-major one-hot: [R, 128(edge), NCHUNK, 128(col)]
        oh_pm = np.ascontiguousarray(oh.transpose(0, 2, 1, 3))
        xc = np.zeros((T, 128, NLP), np.float32)
        xc[:, :, :NL] = x[:, c * NL:(c + 1) * NL, :].transpose(0, 2, 1)
        per_core.append(dict(
            xT_h=xc,
            oh=oh_pm,
            kv_idx=np.stack([_wrap_idx(kv_idx[r]) for r in range(R)]),
            qi_idx=np.stack([_wrap_idx(qi_idx[r]) for r in range(R)]),
        ))

    shared = dict(
        win=np.ascontiguousarray(Win),                        # [3,128,256]
        wq=np.ascontiguousarray(Wq), wa=np.ascontiguousarray(wa_eff),
        wkr=wkr, wvr=wvr, wout=np.ascontiguousarray(Wout),
        ident=np.eye(128, dtype=np.float32),
        bin_b=bcast(b_in), bq_b=bcast(bq), bkr_b=bcast(bkr), bvr_b=bcast(bvr),
        ba_b=bcast(ba_eff), bout_b=bcast(bout),
        lng_b=bcast(ln_g), lnb_b=bcast(ln_b),
    )
    return shared, per_core, meta


def _build(nc, meta, shapes):
    KCH = meta["KCH"]
    NCHUNK = NWIN * KCH
    GC = GWIN * KCH                      # chunks per gather group
    NGRP = NWIN // GWIN
    ub = meta["use_bias"]
    eps_eff = meta["eps_eff"]

    def din(name):
        return nc.dram_tensor(name, shapes[name], I16 if "idx" in name else F32,
                              kind="ExternalInput").ap()

    xT_h = din("xT_h"); oh_d = din("oh")
    kv_idx_d = din("kv_idx"); qi_idx_d = din("qi_idx")
    win_d = din("win"); wq_d = din("wq"); wa_d = din("wa")
    wkr_d = din("wkr"); wvr_d = din("wvr"); wout_d = din("wout")
    ident_d = din("ident")
    bias_d = {k: din(k) for k in
              ("bin_b", "bq_b", "bkr_b", "bvr_b", "ba_b", "bout_b", "lng_b", "lnb_b")}
    y_d = nc.dram_tensor("y", [T, NLP, OUT], F32, kind="ExternalOutput").ap()

    def bc32(ap2d):
        """[128, k] AP -> [128, k, 32] stride-0 broadcast AP."""
        return bass.AP(tensor=ap2d.tensor, offset=ap2d.offset,
                       ap=list(ap2d.ap) + [[0, D]])

    with tile.TileContext(nc) as tc:
        with (
            tc.tile_pool(name="persist", bufs=1) as pp,
            tc.tile_pool(name="wpool", bufs=3) as wp,
            tc.tile_pool(name="stage", bufs=4) as stg,
            tc.tile_pool(name="edge", bufs=2) as ep,
            tc.tile_pool(name="edge1", bufs=1) as ep1,
            tc.tile_pool(name="small", bufs=4) as sp,
            tc.tile_pool(name="psA", bufs=2, space="PSUM") as psA,
            tc.tile_pool(name="psB", bufs=2, space="PSUM") as psB,
            tc.tile_pool(name="psC", bufs=2, space="PSUM") as psC,
            tc.tile_pool(name="dram", bufs=1, space="DRAM") as dp,
        ):
            nc.gpsimd.load_library(mlp)

            ident = pp.tile([128, 128], F32, tag="ident")
            nc.sync.dma_start(ident[:], ident_d)
            h = pp.tile([128, T, NT, H], F32, tag="h")
            agg1 = pp.tile([128, NT, H], F32, tag="agg1")

            kv_loc = dp.tile([NLP, KV_W], F32)
            kv_full = dp.tile([NC * NLP, KV_W], F32)
            q_dram = dp.tile([T, NLP, H], F32)

            def load_w(src_ap):
                """[256, M] dram -> [128, 2, M] sbuf tile."""
                m = src_ap.shape[-1]
                t_ = wp.tile([128, 2, m], F32, tag="w")
                nc.sync.dma_start(t_[:], src_ap.rearrange("(kt kp) m -> kp kt m", kp=128))
                return t_

            def load_bias(src_ap):
                t_ = wp.tile([128, H], F32, tag="bias")
                nc.sync.dma_start(t_[:], src_ap)
                return t_

            # ---- input projection: h[t] = relu(xT^T @ Win + b) ----
            for t in range(T):
                w_in = wp.tile([128, H], F32, tag="w")
                nc.sync.dma_start(w_in[:], win_d[t])
                bt = load_bias(bias_d["bin_b"][t]) if ub["bin_"] else None
                for nt in range(NT):
                    xt = wp.tile([128, 128], F32, tag="xt")
                    nc.sync.dma_start(xt[:], xT_h[t, :, nt * 128:(nt + 1) * 128])
                    ps = psA.tile([128, H], F32)
                    nc.tensor.matmul(ps[:], xt[:],
                                     w_in[:], start=True, stop=True)
                    if bt is not None:
                        nc.vector.tensor_add(ps[:], ps[:], bt[:])
                    nc.scalar.activation(h[:, t, nt, :], ps[:], AF.Relu)

            hT = pp.tile([128, 2, NT, 128], F32, tag="hT")

            def transpose_to(dst3, src2, nt_label):
                """src2 [128, 256] sbuf -> dst3 [128, 2, 128] (feature-major)."""
                for ft in range(2):
                    tp = psB.tile([128, 128], F32)
                    nc.tensor.transpose(tp[:], src2[:, ft * 128:(ft + 1) * 128], ident[:])
                    eng = nc.vector if (nt_label + ft) % 2 else nc.scalar
                    eng.tensor_copy(dst3[:, ft, :], tp[:]) if eng is nc.vector \
                        else nc.scalar.copy(dst3[:, ft, :], tp[:])

            def proj_to_dram(wtile, btile, dst_rows, col0, ncols):
                """out rows = hT^T @ w (+bias) -> dram[dst_rows, col0:col0+ncols]"""
                for nt in range(NT):
                    ps = psA.tile([128, ncols], F32)
                    for kt in range(2):
                        nc.tensor.matmul(ps[:], hT[:, kt, nt, :], wtile[:, kt, :],
                                         start=(kt == 0), stop=(kt == 1))
                    st = stg.tile([128, H], F32, tag="projout")
                    if btile is not None:
                        nc.vector.tensor_add(st[:, :ncols], ps[:], btile[:, :ncols])
                    else:
                        nc.scalar.copy(st[:, :ncols], ps[:])
                    nc.sync.dma_start(
                        dst_rows[nt * 128:(nt + 1) * 128, col0:col0 + ncols],
                        st[:, :ncols])

            for l in range(L):
                # ---- per-type transposes + projections ----
                for t in range(T):
                    for nt in range(NT):
                        transpose_to(hT[:, :, nt, :], h[:, t, nt, :], nt)
                    wq_t = load_w(wq_d[l, t])
                    bq_t = load_bias(bias_d["bq_b"][l, t]) if ub["bq"] else None
                    proj_to_dram(wq_t, bq_t, q_dram[t], 0, H)
                    for r in range(R):
                        if SRC_T[r] != t:
                            continue
                        wk_t = load_w(wkr_d[l, r])
                        bk_t = load_bias(bias_d["bkr_b"][l, r]) if ub["bkr"] else None
                        proj_to_dram(wk_t, bk_t, kv_loc[:], (2 * r) * H, H)
                        wv_t = load_w(wvr_d[l, r])
                        bv_t = load_bias(bias_d["bvr_b"][l, r]) if ub["bkr"] else None
                        proj_to_dram(wv_t, bv_t, kv_loc[:], (2 * r + 1) * H, H)

                nc.gpsimd.collective_compute(
                    "AllGather", OP.bypass,
                    replica_groups=[list(range(NC))],
                    ins=[kv_loc[:].opt()], outs=[kv_full[:].opt()],
                )

                # ---- edge phase;  r order: 0 (t1 agg), 1 (t0), 2 (t1+post), 3 (t2) ----
                s1 = sp.tile([128, NT], F32, tag="s1")
                s2 = sp.tile([128, NT], F32, tag="s2")
                sqs = stg.tile([128, H], F32, tag="sqs")

                def post_edge_window(t, w, agg_norm, wa_t, ba_t):
                    gt = stg.tile([128, H], F32, tag="gelu")
                    nc.scalar.activation(gt[:], agg_norm, AF.Gelu)
                    gT = stg.tile([128, 2, 128], F32, tag="gT")
                    transpose_to(gT[:], gt[:], w)
                    po = psA.tile([128, H], F32)
                    for kt in range(2):
                        nc.tensor.matmul(po[:], gT[:, kt, :], wa_t[:, kt, :],
                                         start=(kt == 0), stop=(kt == 1))
                    if ba_t is not None:
                        nc.vector.tensor_add(po[:], po[:], ba_t[:])
                    # h_pre = o + h (in place), s1 = row sums
                    nc.vector.scalar_tensor_tensor(
                        h[:, t, w, :], po[:], 1.0, h[:, t, w, :],
                        OP.mult, OP.add, accum_out=s1[:, w:w + 1])
                    nc.scalar.activation(sqs[:], h[:, t, w, :], AF.Square,
                                         accum_out=s2[:, w:w + 1])

                def finish_type(t, l):
                    mu = sp.tile([128, NT], F32, tag="mu")
                    inv = sp.tile([128, NT], F32, tag="inv")
                    nmi = sp.tile([128, NT], F32, tag="nmi")
                    nc.vector.tensor_scalar_mul(mu[:], s1[:], 1.0 / H)
                    nc.vector.tensor_scalar_mul(inv[:], s2[:], 1.0 / H)  # mean sq
                    musq = sp.tile([128, NT], F32, tag="musq")
                    nc.vector.tensor_mul(musq[:], mu[:], mu[:])
                    nc.vector.scalar_tensor_tensor(
                        inv[:], inv[:], float(eps_eff[l][t]), musq[:],
                        OP.add, OP.subtract)              # var + eps
                    nc.scalar.activation(inv[:], inv[:], AF.Sqrt)
                    nc.vector.reciprocal(inv[:], inv[:])
                    nc.vector.scalar_tensor_tensor(
                        nmi[:], mu[:], -1.0, inv[:], OP.mult, OP.mult)
                    if ub["lng"] or ub["lnb"]:
                        lng_t = load_bias(bias_d["lng_b"][l, t])
                        lnb_t = load_bias(bias_d["lnb_b"][l, t])
                        for w in range(NT):
                            nc.scalar.activation(
                                h[:, t, w, :], h[:, t, w, :], AF.Identity,
                                bias=nmi[:, w:w + 1], scale=inv[:, w:w + 1])
                            nc.vector.tensor_mul(h[:, t, w, :], h[:, t, w, :], lng_t[:])
                            nc.vector.tensor_add(h[:, t, w, :], h[:, t, w, :], lnb_t[:])
                            nc.scalar.activation(h[:, t, w, :], h[:, t, w, :], AF.Relu)
                    else:
                        for w in range(NT):
                            nc.scalar.activation(
                                h[:, t, w, :], h[:, t, w, :], AF.Relu,
                                bias=nmi[:, w:w + 1], scale=inv[:, w:w + 1])

                for r in (0, 1, 2, 3):
                    dt = DST_T[r]
                    wa_t = ba_t = None
                    if r != 0:
                        wa_t = load_w(wa_d[l, dt])
                        ba_t = load_bias(bias_d["ba_b"][l, dt]) if ub["ba"] else None
                    kvi = sp.tile([128, NIDX_R16(KCH)], I16, tag="kvi")
                    qii = sp.tile([128, NIDX_R16(KCH)], I16, tag="qii")
                    nc.gpsimd.dma_start(kvi[:], kv_idx_d[r])
                    nc.gpsimd.dma_start(qii[:], qi_idx_d[r])
                    for gidx in range(NGRP):
                        ni = GC * 128
                        kvg = ep.tile([128, GC, 2 * H], F32, tag="kvg")
                        qig = ep.tile([128, GC, H], F32, tag="qig")
                        nc.gpsimd.dma_gather(
                            kvg[:], kv_full[:, (2 * r) * H:(2 * r + 2) * H],
                            kvi[:, gidx * (ni // 16):(gidx + 1) * (ni // 16)],
                            ni, ni, 2 * H, elem_step=KV_W)
                        nc.gpsimd.dma_gather(
                            qig[:], q_dram[dt],
                            qii[:, gidx * (ni // 16):(gidx + 1) * (ni // 16)],
                            ni, ni, H)
                        ohg = ep.tile([128, GC, 128], F32, tag="ohg")
                        nc.sync.dma_start(ohg[:], oh_d[r, :, gidx * GC:(gidx + 1) * GC, :])
                        msg = ep1.tile([128, GC, H + HEADS], F32, tag="msg")
                        lg = sp.tile([128, GC, HEADS], F32, tag="lg")
                        nc.vector.tensor_mul(msg[:, :, 0:H], qig[:], kvg[:, :, 0:H])
                        nc.vector.tensor_reduce(
                            lg[:], msg[:, :, 0:H].rearrange("p g (hh dd) -> p g hh dd", dd=D),
                            mybir.AxisListType.X, OP.add)
                        nc.scalar.activation(msg[:, :, H:H + HEADS], lg[:], AF.Exp)
                        nc.vector.tensor_mul(
                            msg[:, :, 0:H].rearrange("p g (hh dd) -> p g hh dd", dd=D),
                            kvg[:, :, H:2 * H].rearrange("p g (hh dd) -> p g hh dd", dd=D),
                            bc32(msg[:, :, H:H + HEADS]))
                        for wi in range(GWIN):
                            w = gidx * GWIN + wi
                            pw = psC.tile([128, H + HEADS], F32)
                            for kc in range(KCH):
                                nc.tensor.matmul(
                                    pw[:], ohg[:, wi * KCH + kc, :],
                                    msg[:, wi * KCH + kc, :],
                                    start=(kc == 0), stop=(kc == KCH - 1))
                            rec = sp.tile([128, HEADS], F32, tag="rec")
                            # +1e-30: degree-0 dst nodes have sum 0; keep 0*recip = 0
                            nc.vector.tensor_scalar_add(rec[:], pw[:, H:H + HEADS], 1e-30)
                            nc.vector.reciprocal(rec[:], rec[:])
                            if r == 0:
                                nc.vector.tensor_mul(
                                    agg1[:, w, :].rearrange("p (hh dd) -> p hh dd", dd=D),
                                    pw[:, 0:H].rearrange("p (hh dd) -> p hh dd", dd=D),
                                    bc32(rec[:]))
                            else:
                                an = stg.tile([128, H], F32, tag="aggn")
                                nc.vector.tensor_mul(
                                    an[:].rearrange("p (hh dd) -> p hh dd", dd=D),
                                    pw[:, 0:H].rearrange("p (hh dd) -> p hh dd", dd=D),
                                    bc32(rec[:]))
                                if r == 2:
                                    nc.vector.tensor_add(an[:], an[:], agg1[:, w, :])
                                post_edge_window(dt, w, an[:], wa_t, ba_t)
                    if r != 0:
                        finish_type(dt, l)

            # ---- output projection ----
            wo = load_w(wout_d)
            bo = load_bias(bias_d["bout_b"]) if ub["bout"] else None
            for t in range(T):
                for nt in range(NT):
                    transpose_to(hT[:, :, nt, :], h[:, t, nt, :], nt)
                    ps = psA.tile([128, OUT], F32)
                    for kt in range(2):
                        nc.tensor.matmul(ps[:], hT[:, kt, nt, :], wo[:, kt, :OUT],
                                         start=(kt == 0), stop=(kt == 1))
                    st = stg.tile([128, OUT], F32, tag="yout")
                    if bo is not None:
                        nc.vector.tensor_add(st[:], ps[:], bo[:, :OUT])
                    else:
                        nc.scalar.copy(st[:], ps[:])
                    nc.sync.dma_start(y_d[t, nt * 128:(nt + 1) * 128, :], st[:])
    nc.compile()
    return nc


def NIDX_R16(KCH):
    return NWIN * KCH * 128 // 16


def kernel(**inputs):
    shared, per_core, meta = _preprocess(inputs)
    shapes = {k: list(v.shape) for k, v in {**shared, **per_core[0]}.items()}
    nc = bacc.Bacc("TRN2", target_bir_lowering=False, debug=False, num_devices=NC)
    nc = _build(nc, meta, shapes)
    in_maps = [{**shared, **per_core[c]} for c in range(NC)]
    res = run_bass_kernel_spmd(nc, in_maps, core_ids=list(range(NC)))
    y = np.concatenate([res.results[c]["y"][:, :NL, :] for c in range(NC)], axis=1)
    return y.astype(np.float32)


if __name__ == "__main__":
    import reference
    inputs = {k: np.asarray(v) for k, v in reference.setup_inputs().items()}
    out = kernel(**inputs)
    exp = np.asarray(reference.reference(**inputs))
    err = np.abs(out - exp).max() / np.abs(exp).max()
    print("Relative error:", err)



# revision 9
# speedup vs baseline: 37.6017x; 37.6017x over previous
"""HGT link predictor on 8 trn2 NeuronCores — v2.

Sharding: nodes split 8 ways per type with host-side load balancing so every
(core, 128-dst-window) holds <= KCH*128 edges per relation (KCH=3 typical).
Params replicated. Per layer: raw k/v projections for source types (0,1) in
bf16 -> per-type Shared-output AllGather (overlapped with other projections /
previous edge phases) -> per-relation q projections with a_rel folded in
(logits = q_r . k_raw) -> per-edge dma_gather of k|v and q_r (bf16) ->
logits via multiply+segmented reduce -> exp -> scatter-add via one-hot
matmuls into PSUM (softmax denominator rides as columns 256..263) ->
normalize -> per-relation m_rel block-diag post-transform -> gelu -> output
linear (skip gate folded) -> residual+LayerNorm+relu. All matmuls bf16 with
fp32 PSUM accumulation; LayerNorm statistics in fp32.
"""
import math
import numpy as np

import concourse.bacc as bacc
import concourse.bass as bass
import concourse.mybir as mybir
import concourse.tile as tile
from concourse.bass_utils import run_bass_kernel_spmd
from concourse.library_config import mlp

F32 = mybir.dt.float32
BF16 = mybir.dt.bfloat16
BF16_NP = mybir.dt.np(mybir.dt.bfloat16)
I16 = mybir.dt.int16
AF = mybir.ActivationFunctionType
OP = mybir.AluOpType

T, R, L = 3, 4, 2
H, HEADS, D, FIN, OUT = 256, 8, 32, 128, 128
SRC_T = (0, 1, 1, 1)
DST_T = (1, 0, 1, 2)
LN_EPS = 1e-5
NC = 8
N = 20000
NL = N // NC          # 2500 real local nodes per type
NT = 20               # node tiles of 128
NLP = NT * 128        # 2560 padded local nodes
NWIN = NT             # dst windows of 128 local slots
NBINS = NC * NWIN     # 160
CAP = N // NBINS      # 125 real nodes per window
GWIN = 4              # windows per gather group


def _block_diag(a):
    """a: [HEADS, D, D] -> [H, H] block diagonal."""
    out = np.zeros((H, H), np.float32)
    for h in range(HEADS):
        out[h * D:(h + 1) * D, h * D:(h + 1) * D] = a[h]
    return out


def _wrap_idx(idx):
    """idx [M] -> [128, M//16] int16 wrapped in 16 partitions, replicated."""
    m = idx.shape[0]
    assert m % 16 == 0
    w = np.zeros((128, m // 16), np.int16)
    w[:16] = idx.astype(np.int16).reshape(m // 16, 16).T
    for rep in range(1, 8):
        w[16 * rep:16 * rep + 16] = w[:16]
    return w


def _balance(deg_dims):
    """deg_dims: [K, N] per-relation degrees. Greedy best-fit-decreasing into
    NBINS bins of CAP nodes, minimizing the max per-dimension bin load.
    Returns core_of [N], slot_of [N]."""
    n = deg_dims.shape[1]
    order = np.argsort(-deg_dims.sum(0), kind="stable")
    loads = np.zeros((deg_dims.shape[0], NBINS), np.float64)
    counts = np.zeros(NBINS, np.int64)
    binof = np.empty(n, np.int64)
    for node in order:
        cand = (loads + deg_dims[:, node:node + 1]).max(0) + 1e-6 * loads.sum(0)
        cand[counts >= CAP] = np.inf
        b = int(np.argmin(cand))
        binof[node] = b
        loads[:, b] += deg_dims[:, node]
        counts[b] += 1
    core_of = binof // NWIN
    win = binof % NWIN
    # slot within window: dense by order of node id
    slot_of = np.empty(n, np.int64)
    for b in range(NBINS):
        nodes = np.where(binof == b)[0]
        w = b % NWIN
        slot_of[nodes] = 128 * w + np.arange(len(nodes))
    return core_of, slot_of


def _preprocess(inputs):
    x = np.asarray(inputs["x"], np.float32)
    edge_index = np.asarray(inputs["edge_index"])
    Win = np.asarray(inputs["Win"], np.float32)
    b_in = np.asarray(inputs["b_in"], np.float32)
    Wk = np.asarray(inputs["Wk"], np.float32); bk = np.asarray(inputs["bk"], np.float32)
    Wq = np.asarray(inputs["Wq"], np.float32); bq = np.asarray(inputs["bq"], np.float32)
    Wv = np.asarray(inputs["Wv"], np.float32); bv = np.asarray(inputs["bv"], np.float32)
    Wa = np.asarray(inputs["Wa"], np.float32); ba = np.asarray(inputs["ba"], np.float32)
    skip = np.asarray(inputs["skip"], np.float32)
    a_rel = np.asarray(inputs["a_rel"], np.float32)
    m_rel = np.asarray(inputs["m_rel"], np.float32)
    p_rel = np.asarray(inputs["p_rel"], np.float32)
    ln_g = np.asarray(inputs["ln_g"], np.float32)
    ln_b = np.asarray(inputs["ln_b"], np.float32)
    Wout = np.asarray(inputs["Wout"], np.float32)
    bout = np.asarray(inputs["bout"], np.float32)

    meta = {}
    inv_sqrt_d = 1.0 / math.sqrt(D)

    # folded weights ------------------------------------------------------
    # q-side: logits = (q @ MA) . k_raw with MA = blockdiag(a_h^T * p_h/sqrt(D))
    wqr = np.zeros((L, R, H, H), np.float32); bqr = np.zeros((L, R, H), np.float32)
    # v-side post-transform: agg_out = agg_raw @ blockdiag(m_h); store the two
    # diagonal 128x128 quadrants (off-diagonal quadrants are zero).
    mq = np.zeros((L, R, 2, 128, 128), np.float32)
    for l in range(L):
        for r in range(R):
            dt = DST_T[r]
            ma = _block_diag(np.transpose(a_rel[l, r], (0, 2, 1))
                             * (p_rel[l, r] * inv_sqrt_d)[:, None, None])
            wqr[l, r] = Wq[l, dt] @ ma
            bqr[l, r] = bq[l, dt] @ ma
            mv = _block_diag(m_rel[l, r])
            mq[l, r, 0] = mv[0:128, 0:128]
            mq[l, r, 1] = mv[128:256, 128:256]
    beta = 1.0 / (1.0 + np.exp(-skip))          # [L, T]
    g = beta / (2.0 - beta)
    wa_eff = Wa * g[:, :, None, None]
    ba_eff = ba * g[:, :, None]
    meta["eps_eff"] = (LN_EPS / (2.0 - beta) ** 2).tolist()

    meta["use_bias"] = dict(
        bin_=bool(np.any(b_in)), bqr=bool(np.any(bqr)),
        bk=bool(np.any(bk)) or bool(np.any(bv)),
        ba=bool(np.any(ba_eff)), bout=bool(np.any(bout)),
        lng=not np.allclose(ln_g, 1.0), lnb=bool(np.any(ln_b)),
    )

    def bcast(v):
        # [..., F] -> [..., 128, F]: per-feature vectors replicated across partitions
        return np.ascontiguousarray(
            np.broadcast_to(v[..., None, :], v.shape[:-1] + (128, v.shape[-1])))

    # node balancing ------------------------------------------------------
    # per dst type, balance windows across the relations that aggregate there
    deg = np.zeros((R, N), np.int64)
    for r in range(R):
        np.add.at(deg[r], edge_index[r, 1], 1)
    rels_of_t = [[r for r in range(R) if DST_T[r] == t] for t in range(T)]
    core_of = np.zeros((T, N), np.int64)
    slot_of = np.zeros((T, N), np.int64)
    for t in range(T):
        core_of[t], slot_of[t] = _balance(deg[rels_of_t[t]])
    meta["core_of"] = core_of
    meta["slot_of"] = slot_of

    # edge partitioning ---------------------------------------------------
    win_edges = [[None] * R for _ in range(NC)]   # [c][r] -> list of (src, dslot) per win
    kch_need = 1
    for c in range(NC):
        for r in range(R):
            st, dt = SRC_T[r], DST_T[r]
            src = edge_index[r, 0].astype(np.int64)
            dst = edge_index[r, 1].astype(np.int64)
            sel = core_of[dt][dst] == c
            s, d = src[sel], slot_of[dt][dst[sel]]
            o = np.argsort(d, kind="stable")
            s, d = s[o], d[o]
            wins = []
            for w in range(NWIN):
                m = (d // 128) == w
                sw, dw = s[m], d[m]
                kch_need = max(kch_need, (len(sw) + 127) // 128)
                wins.append((sw, dw))
            win_edges[c][r] = wins
    KCH = kch_need
    meta["KCH"] = KCH
    NCHUNK = NWIN * KCH
    NIDX_R = NCHUNK * 128

    xT = np.ascontiguousarray(x.transpose(0, 2, 1))  # [T, FIN, N]
    per_core = []
    for c in range(NC):
        oh = np.zeros((R, NCHUNK, 128, 128), np.float32)
        kv_idx = np.zeros((R, NIDX_R), np.int64)
        qi_idx = np.zeros((R, NIDX_R), np.int64)
        for r in range(R):
            st = SRC_T[r]
            for w in range(NWIN):
                sw, dw = win_edges[c][r][w]
                ne = len(sw)
                base = w * KCH * 128
                kv_idx[r, base:base + ne] = core_of[st][sw] * NLP + slot_of[st][sw]
                qi_idx[r, base:base + ne] = dw
                ch = base // 128 + np.arange(ne) // 128
                oh[r, ch, np.arange(ne) % 128, dw - w * 128] = 1.0
        # partition-major one-hot: [R, 128(edge), NCHUNK, 128(col)]
        oh_pm = np.ascontiguousarray(oh.transpose(0, 2, 1, 3))
        xc = np.zeros((T, FIN, NLP), np.float32)
        for t in range(T):
            idx = np.where(core_of[t] == c)[0]
            xc[t][:, slot_of[t][idx]] = xT[t][:, idx]
        per_core.append(dict(
            xT_h=xc.astype(BF16_NP),
            oh=oh_pm.astype(BF16_NP),
            kv_idx=np.stack([_wrap_idx(kv_idx[r]) for r in range(R)]),
            qi_idx=np.stack([_wrap_idx(qi_idx[r]) for r in range(R)]),
        ))

    def b16(a):
        return np.ascontiguousarray(a).astype(BF16_NP)

    shared = dict(
        win=b16(Win),                                      # [3,128,256]
        wk=b16(Wk), wv=b16(Wv),                            # raw [L,T,256,256]
        wqr=b16(wqr), wa=b16(wa_eff), mq=b16(mq),
        wout=b16(Wout),
        ident=np.eye(128, dtype=np.float32).astype(BF16_NP),
        bin_b=bcast(b_in), bqr_b=bcast(bqr),
        bk_b=bcast(bk), bv_b=bcast(bv),
        ba_b=bcast(ba_eff), bout_b=bcast(bout),
        lng_b=bcast(ln_g), lnb_b=bcast(ln_b),
    )
    return shared, per_core, meta


def NIDX16(KCH):
    return NWIN * KCH * 128 // 16


def _build(nc, meta, shapes):
    KCH = meta["KCH"]
    NCHUNK = NWIN * KCH
    GC = GWIN * KCH                      # chunks per gather group
    NGRP = NWIN // GWIN
    ub = meta["use_bias"]
    eps_eff = meta["eps_eff"]

    def din(name, dtype=BF16):
        if "idx" in name:
            dtype = I16
        return nc.dram_tensor(name, shapes[name], dtype, kind="ExternalInput").ap()

    xT_h = din("xT_h"); oh_d = din("oh")
    kv_idx_d = din("kv_idx"); qi_idx_d = din("qi_idx")
    win_d = din("win"); wk_d = din("wk"); wv_d = din("wv")
    wqr_d = din("wqr"); wa_d = din("wa"); mq_d = din("mq")
    wout_d = din("wout")
    ident_d = din("ident")
    bias_d = {k: din(k, F32) for k in
              ("bin_b", "bqr_b", "bk_b", "bv_b", "ba_b", "bout_b", "lng_b", "lnb_b")}
    y_d = nc.dram_tensor("y", [T, NLP, OUT], F32, kind="ExternalOutput").ap()

    def bc32(ap2d):
        """[..., k] AP -> [..., k, 32] stride-0 broadcast AP."""
        return bass.AP(tensor=ap2d.tensor, offset=ap2d.offset,
                       ap=list(ap2d.ap) + [[0, D]])

    with tile.TileContext(nc) as tc:
        with (
            tc.tile_pool(name="persist", bufs=1) as pp,
            tc.tile_pool(name="hTp", bufs=3) as hTp,
            tc.tile_pool(name="wpool", bufs=3) as wp,
            tc.tile_pool(name="stage", bufs=4) as stg,
            tc.tile_pool(name="edge", bufs=2) as ep,
            tc.tile_pool(name="edge1", bufs=2) as ep1,
            tc.tile_pool(name="small", bufs=4) as sp,
            tc.tile_pool(name="psA", bufs=2, space="PSUM") as psA,
            tc.tile_pool(name="psB", bufs=2, space="PSUM") as psB,
            tc.tile_pool(name="psC", bufs=2, space="PSUM") as psC,
            tc.tile_pool(name="psD", bufs=2, space="PSUM") as psD,
            tc.tile_pool(name="dram", bufs=1, space="DRAM") as dp,
        ):
            nc.gpsimd.load_library(mlp)

            ident = pp.tile([128, 128], BF16, tag="ident")
            nc.sync.dma_start(ident[:], ident_d)
            h = pp.tile([128, T, NT, H], BF16, tag="h")
            agg1 = pp.tile([128, NT, H], BF16, tag="agg1")
            # edge indices: identical across layers, load once
            kvi = pp.tile([128, R, NIDX16(KCH)], I16, tag="kvi")
            qii = pp.tile([128, R, NIDX16(KCH)], I16, tag="qii")
            for r in range(R):
                nc.sync.dma_start(kvi[:, r, :], kv_idx_d[r])
                nc.sync.dma_start(qii[:, r, :], qi_idx_d[r])

            # per-layer double-buffered kv/q tables (no cross-layer WAR)
            kv_loc = [[dp.tile([NLP, 2 * H], BF16, name=f"kv_loc_{l}_{t}")
                       for t in range(2)] for l in range(L)]
            kv_full = [[dp.tile([NC * NLP, 2 * H], BF16,
                                name=f"kv_full_{l}_{t}")
                        for t in range(2)] for l in range(L)]
            qr_dram = [[dp.tile([NLP, H], BF16, name=f"qr_{l}_{r}")
                        for r in range(R)] for l in range(L)]

            def load_w(src_ap):
                """[256, M] dram -> [128, 2, M] sbuf tile."""
                m = src_ap.shape[-1]
                t_ = wp.tile([128, 2, m], BF16, tag="w")
                nc.sync.dma_start(t_[:], src_ap.rearrange("(kt kp) m -> kp kt m", kp=128))
                return t_

            def load_bias(src_ap):
                t_ = wp.tile([128, H], F32, tag="bias")
                nc.sync.dma_start(t_[:], src_ap)
                return t_

            # ---- input projection: h[t] = relu(xT^T @ Win + b) ----
            for t in range(T):
                w_in = wp.tile([128, H], BF16, tag="w")
                nc.sync.dma_start(w_in[:], win_d[t])
                bt = load_bias(bias_d["bin_b"][t]) if ub["bin_"] else None
                for nt in range(NT):
                    xt = wp.tile([128, 128], BF16, tag="xt")
                    nc.sync.dma_start(xt[:], xT_h[t, :, nt * 128:(nt + 1) * 128])
                    ps = psA.tile([128, H], F32)
                    nc.tensor.matmul(ps[:], xt[:], w_in[:], start=True, stop=True)
                    if bt is not None:
                        nc.vector.tensor_add(ps[:], ps[:], bt[:])
                    nc.scalar.activation(h[:, t, nt, :], ps[:], AF.Relu)

            def transposes(t):
                """h[:, t] -> fresh feature-major tile [128, 2, NT, 128]."""
                hT = hTp.tile([128, 2, NT, 128], BF16, tag="hT")
                for nt in range(NT):
                    for ft in range(2):
                        tp = psB.tile([128, 128], BF16)
                        nc.tensor.transpose(
                            tp[:], h[:, t, nt, ft * 128:(ft + 1) * 128], ident[:])
                        if (nt + ft) % 2:
                            nc.vector.tensor_copy(hT[:, ft, nt, :], tp[:])
                        else:
                            nc.scalar.copy(hT[:, ft, nt, :], tp[:])
                return hT

            def kv_proj_ag(l, t, hT):
                """raw k,v projections for type t -> kv_loc[l][t] -> AllGather."""
                wk_t = load_w(wk_d[l, t])
                wv_t = load_w(wv_d[l, t])
                bk_t = load_bias(bias_d["bk_b"][l, t]) if ub["bk"] else None
                bv_t = load_bias(bias_d["bv_b"][l, t]) if ub["bk"] else None
                for nt in range(NT):
                    st_ = stg.tile([128, 2 * H], BF16, tag="kvstage")
                    for j, (w_t, b_t) in enumerate(((wk_t, bk_t), (wv_t, bv_t))):
                        ps = psA.tile([128, H], F32)
                        for kt in range(2):
                            nc.tensor.matmul(ps[:], hT[:, kt, nt, :], w_t[:, kt, :],
                                             start=(kt == 0), stop=(kt == 1))
                        if b_t is not None:
                            nc.vector.tensor_add(ps[:], ps[:], b_t[:])
                            nc.scalar.copy(st_[:, j * H:(j + 1) * H], ps[:])
                        elif j:
                            nc.vector.tensor_copy(st_[:, j * H:(j + 1) * H], ps[:])
                        else:
                            nc.scalar.copy(st_[:, j * H:(j + 1) * H], ps[:])
                    nc.sync.dma_start(
                        kv_loc[l][t][nt * 128:(nt + 1) * 128, :], st_[:])
                nc.gpsimd.collective_compute(
                    "AllGather", OP.bypass,
                    replica_groups=[list(range(NC))],
                    ins=[kv_loc[l][t][:].opt()], outs=[kv_full[l][t][:].opt()],
                )

            def q_proj(l, r, hT):
                """folded q projection for relation r -> qr_dram[l][r]."""
                wq_t = load_w(wqr_d[l, r])
                bq_t = load_bias(bias_d["bqr_b"][l, r]) if ub["bqr"] else None
                for nt in range(NT):
                    ps = psA.tile([128, H], F32)
                    for kt in range(2):
                        nc.tensor.matmul(ps[:], hT[:, kt, nt, :], wq_t[:, kt, :],
                                         start=(kt == 0), stop=(kt == 1))
                    st_ = stg.tile([128, H], BF16, tag="qstage")
                    if bq_t is not None:
                        nc.vector.tensor_add(ps[:], ps[:], bq_t[:])
                        nc.scalar.copy(st_[:], ps[:])
                    else:
                        nc.scalar.copy(st_[:], ps[:])
                    nc.sync.dma_start(
                        qr_dram[l][r][nt * 128:(nt + 1) * 128, :], st_[:])

            s1 = pp.tile([128, NT], F32, tag="s1")
            s2 = pp.tile([128, NT], F32, tag="s2")
            sqs = pp.tile([128, H], F32, tag="sqs")

            def transpose2(dst3, src, lbl):
                """src [128, 256] (sbuf) -> dst3 [128, 2, 128] bf16."""
                for ft in range(2):
                    tp = psB.tile([128, 128], BF16)
                    nc.tensor.transpose(tp[:], src[:, ft * 128:(ft + 1) * 128], ident[:])
                    if (lbl + ft) % 2:
                        nc.vector.tensor_copy(dst3[:, ft, :], tp[:])
                    else:
                        nc.scalar.copy(dst3[:, ft, :], tp[:])

            def edge_phase(l, r):
                st_t, dt = SRC_T[r], DST_T[r]
                wa_t = ba_t = None
                if r != 0:
                    wa_t = load_w(wa_d[l, dt])
                    ba_t = load_bias(bias_d["ba_b"][l, dt]) if ub["ba"] else None
                mq_t = load_w(mq_d[l, r].rearrange("q a b -> (q a) b"))
                for gidx in range(NGRP):
                    ni = GC * 128
                    kvg = ep.tile([128, GC, 2 * H], BF16, tag="kvg")
                    qig = ep.tile([128, GC, H], BF16, tag="qig")
                    nc.gpsimd.dma_gather(
                        kvg[:], kv_full[l][st_t][:],
                        kvi[:, r, gidx * (ni // 16):(gidx + 1) * (ni // 16)],
                        ni, ni, 2 * H)
                    nc.gpsimd.dma_gather(
                        qig[:], qr_dram[l][r][:],
                        qii[:, r, gidx * (ni // 16):(gidx + 1) * (ni // 16)],
                        ni, ni, H)
                    ohg = ep.tile([128, GC, 128], BF16, tag="ohg")
                    nc.sync.dma_start(ohg[:], oh_d[r, :, gidx * GC:(gidx + 1) * GC, :])
                    prod = ep1.tile([128, GC, H], BF16, tag="prod")
                    lg = sp.tile([128, GC, HEADS], F32, tag="lg")
                    msg = ep1.tile([128, GC, H + HEADS], BF16, tag="msg")
                    nc.vector.tensor_mul(prod[:], qig[:], kvg[:, :, 0:H])
                    nc.vector.tensor_reduce(
                        lg[:], prod[:].rearrange("p g (hh dd) -> p g hh dd", dd=D),
                        mybir.AxisListType.X, OP.add)
                    nc.scalar.activation(msg[:, :, H:H + HEADS], lg[:], AF.Exp)
                    nc.vector.tensor_mul(
                        msg[:, :, 0:H].rearrange("p g (hh dd) -> p g hh dd", dd=D),
                        kvg[:, :, H:2 * H].rearrange("p g (hh dd) -> p g hh dd", dd=D),
                        bc32(msg[:, :, H:H + HEADS]))
                    for wi in range(GWIN):
                        w = gidx * GWIN + wi
                        pw = psC.tile([128, H + HEADS], F32)
                        for kc in range(KCH):
                            nc.tensor.matmul(
                                pw[:], ohg[:, wi * KCH + kc, :],
                                msg[:, wi * KCH + kc, :],
                                start=(kc == 0), stop=(kc == KCH - 1))
                        rec = sp.tile([128, HEADS], F32, tag="rec")
                        # +1e-30: degree-0 dst slots have sum 0; keep 0*recip = 0
                        nc.vector.tensor_scalar_add(rec[:], pw[:, H:H + HEADS], 1e-30)
                        nc.vector.reciprocal(rec[:], rec[:])
                        an = stg.tile([128, H], BF16, tag="an")
                        nc.vector.tensor_mul(
                            an[:].rearrange("p (hh dd) -> p hh dd", dd=D),
                            pw[:, 0:H].rearrange("p (hh dd) -> p hh dd", dd=D),
                            bc32(rec[:]))
                        anT = stg.tile([128, 2, 128], BF16, tag="anT")
                        transpose2(anT, an, w)
                        ps2 = psD.tile([128, H], F32, tag="pp")
                        nc.tensor.matmul(ps2[:, 0:128], anT[:, 0, :], mq_t[:, 0, :],
                                         start=True, stop=True)
                        nc.tensor.matmul(ps2[:, 128:256], anT[:, 1, :], mq_t[:, 1, :],
                                         start=True, stop=True)
                        if r == 0:
                            if w % 2:
                                nc.vector.tensor_copy(agg1[:, w, :], ps2[:])
                            else:
                                nc.scalar.copy(agg1[:, w, :], ps2[:])
                            continue
                        if r == 2:
                            nc.vector.tensor_add(ps2[:], ps2[:], agg1[:, w, :])
                        gt = stg.tile([128, H], BF16, tag="gelu")
                        nc.scalar.activation(gt[:], ps2[:], AF.Gelu)
                        gT = stg.tile([128, 2, 128], BF16, tag="gT")
                        transpose2(gT, gt, w + 1)
                        po = psD.tile([128, H], F32, tag="pp")
                        for kt in range(2):
                            nc.tensor.matmul(po[:], gT[:, kt, :], wa_t[:, kt, :],
                                             start=(kt == 0), stop=(kt == 1))
                        if ba_t is not None:
                            nc.vector.tensor_add(po[:], po[:], ba_t[:])
                        # h_pre = o + h (in place), s1 = row sums
                        nc.vector.scalar_tensor_tensor(
                            h[:, dt, w, :], po[:], 1.0, h[:, dt, w, :],
                            OP.mult, OP.add, accum_out=s1[:, w:w + 1])
                        nc.scalar.activation(sqs[:], h[:, dt, w, :], AF.Square,
                                             accum_out=s2[:, w:w + 1])

            def finish_type(l, t):
                mu = sp.tile([128, NT], F32, tag="mu")
                inv = sp.tile([128, NT], F32, tag="inv")
                nmi = sp.tile([128, NT], F32, tag="nmi")
                nc.vector.tensor_scalar_mul(mu[:], s1[:], 1.0 / H)
                nc.vector.tensor_scalar_mul(inv[:], s2[:], 1.0 / H)  # mean sq
                musq = sp.tile([128, NT], F32, tag="musq")
                nc.vector.tensor_mul(musq[:], mu[:], mu[:])
                nc.vector.scalar_tensor_tensor(
                    inv[:], inv[:], float(eps_eff[l][t]), musq[:],
                    OP.add, OP.subtract)              # var + eps
                nc.scalar.activation(inv[:], inv[:], AF.Sqrt)
                nc.vector.reciprocal(inv[:], inv[:])
                nc.vector.scalar_tensor_tensor(
                    nmi[:], mu[:], -1.0, inv[:], OP.mult, OP.mult)
                if ub["lng"] or ub["lnb"]:
                    lng_t = load_bias(bias_d["lng_b"][l, t])
                    lnb_t = load_bias(bias_d["lnb_b"][l, t])
                    for w in range(NT):
                        nc.scalar.activation(
                            h[:, t, w, :], h[:, t, w, :], AF.Identity,
                            bias=nmi[:, w:w + 1], scale=inv[:, w:w + 1])
                        nc.vector.tensor_mul(h[:, t, w, :], h[:, t, w, :], lng_t[:])
                        nc.vector.tensor_add(h[:, t, w, :], h[:, t, w, :], lnb_t[:])
                        nc.scalar.activation(h[:, t, w, :], h[:, t, w, :], AF.Relu)
                else:
                    for w in range(NT):
                        nc.scalar.activation(
                            h[:, t, w, :], h[:, t, w, :], AF.Relu,
                            bias=nmi[:, w:w + 1], scale=inv[:, w:w + 1])

            # ---- layer 0 projections + collectives ----
            hT0 = transposes(0)
            kv_proj_ag(0, 0, hT0)
            hT1 = transposes(1)
            kv_proj_ag(0, 1, hT1)
            hT2 = transposes(2)
            hT_by_t = {0: hT0, 1: hT1, 2: hT2}
            for r in range(R):
                q_proj(0, r, hT_by_t[DST_T[r]])

            # ---- layer 0 edge phases, interleaved with layer-1 projections ----
            edge_phase(0, 0)
            edge_phase(0, 1)
            finish_type(0, 0)
            hT0b = transposes(0)
            kv_proj_ag(1, 0, hT0b)
            q_proj(1, 1, hT0b)
            edge_phase(0, 2)
            finish_type(0, 1)
            hT1b = transposes(1)
            kv_proj_ag(1, 1, hT1b)
            q_proj(1, 0, hT1b)
            q_proj(1, 2, hT1b)
            edge_phase(0, 3)
            finish_type(0, 2)
            hT2b = transposes(2)
            q_proj(1, 3, hT2b)

            # ---- layer 1 edge phases ----
            edge_phase(1, 0)
            edge_phase(1, 1)
            finish_type(1, 0)
            edge_phase(1, 2)
            finish_type(1, 1)
            edge_phase(1, 3)
            finish_type(1, 2)

            # ---- output projection ----
            wo = load_w(wout_d)
            bo = load_bias(bias_d["bout_b"]) if ub["bout"] else None
            for t in range(T):
                hTo = transposes(t)
                for nt in range(NT):
                    ps = psA.tile([128, OUT], F32)
                    for kt in range(2):
                        nc.tensor.matmul(ps[:], hTo[:, kt, nt, :], wo[:, kt, :OUT],
                                         start=(kt == 0), stop=(kt == 1))
                    st_ = stg.tile([128, OUT], F32, tag="yout")
                    if bo is not None:
                        nc.vector.tensor_add(st_[:], ps[:], bo[:, :OUT])
                    else:
                        nc.scalar.copy(st_[:], ps[:])
                    nc.sync.dma_start(y_d[t, nt * 128:(nt + 1) * 128, :], st_[:])
    nc.compile()
    return nc


def kernel(**inputs):
    shared, per_core, meta = _preprocess(inputs)
    shapes = {k: list(v.shape) for k, v in {**shared, **per_core[0]}.items()}
    nc = bacc.Bacc("TRN2", target_bir_lowering=False, debug=False, num_devices=NC)
    nc = _build(nc, meta, shapes)
    in_maps = [{**shared, **per_core[c]} for c in range(NC)]
    res = run_bass_kernel_spmd(nc, in_maps, core_ids=list(range(NC)))
    core_of, slot_of = meta["core_of"], meta["slot_of"]
    y = np.empty((T, N, OUT), np.float32)
    for c in range(NC):
        yc = np.asarray(res.results[c]["y"], np.float32)
        for t in range(T):
            idx = np.where(core_of[t] == c)[0]
            y[t, idx] = yc[t, slot_of[t][idx]]
    return y


if __name__ == "__main__":
    import reference
    inputs = {k: np.asarray(v) for k, v in reference.setup_inputs().items()}
    out = kernel(**inputs)
    exp = np.asarray(reference.reference(**inputs))
    err = np.abs(out - exp).max() / np.abs(exp).max()
    print("Relative error:", err)
